# revision 12
# baseline (speedup 1.0000x reference)
"""Trainium2 Bass kernel for nn_CHESHIRE (hypergraph GNN message passing).

Strategy (hyperedge-parallel across the 8 cores):
  * Clique Laplacian over 8-node cliques collapses the K=3 Chebyshev conv to
    out = x_gn @ Wx + gsum(x_gn) @ Wg with host-folded weight combos; the
    GraphNorm affine is folded into the same matmuls, with gn_weight/
    gn_mean_scale folded into the weight matrices host-side so the per-edge
    scale is just ex = rsqrt(var+eps).
  * Node encodings are computed once per core and stored to DRAM as an fp16
    [node, x] table (256B rows); incidence rows are fetched with SWDGE
    dma_gather (transpose=True), which lands the data feature-major directly
    (no PE transposes), one 512-idx gather per member plane, round-robined
    over 4 SWDGE queues so descriptor rings never block back-to-back.
  * Per-edge sums (g8) accumulate over the 8 member planes in PSUM via
    identity matmuls; the stats chain runs in fp32 (variance cancellation);
    q8/ssq/max/min pool via fp16 tensor-tensor trees.
  * The per-edge ChebConv constant C is prefilled into PSUM (identity matmul)
    so the conv matmul accumulates on top; clip is applied after the pooling
    trees (exact: clip is monotone, and min(z^2,1) == clip(z)^2).
"""

import sys

sys.path.insert(0, "/opt/trn_rl_repo")

import numpy as np

import concourse.bacc as bacc
import concourse.bass as bass
import concourse.mybir as mybir
from concourse import tile
from concourse.bass_utils import run_bass_kernel_spmd

F16 = mybir.dt.float16
F32 = mybir.dt.float32
I16 = mybir.dt.int16
AF = mybir.ActivationFunctionType
OP = mybir.AluOpType

# Problem constants (hardcoded per contract).
N, F, EMB, CONV = 2000, 256, 128, 128
E, S = 20000, 8
NCORES = 8
ECORE = E // NCORES          # 2500
NBLK = 5
LB = 512                     # edges per block
EPAD = NBLK * LB             # 2560
COLS = S * LB                # 4096 gathered columns per block
NIDXCOL = EPAD * S // 16     # 1280 int16 idx columns per core
NPAD = 2048                  # padded node count
EPS = 1e-5

_CACHE = {}


def _build_program():
    nc = bacc.Bacc(None, target_bir_lowering=False, debug=False,
                   num_swdge_queues=4)

    featT_d = nc.dram_tensor("featT", [F, NPAD], F16, kind="ExternalInput")
    wenc_d = nc.dram_tensor("wenc", [F, EMB], F16, kind="ExternalInput")
    wx_d = nc.dram_tensor("wx", [EMB, CONV], F16, kind="ExternalInput")
    wcs_d = nc.dram_tensor("wcs", [EMB, CONV], F16, kind="ExternalInput")
    wo_d = nc.dram_tensor("wo", [CONV, 4], F16, kind="ExternalInput")
    eyef_d = nc.dram_tensor("eyef", [128, 128], F16, kind="ExternalInput")
    vecs_d = nc.dram_tensor("vecs", [128, 8], F32, kind="ExternalInput")
    idx_d = nc.dram_tensor("idx16", [128, NIDXCOL], I16, kind="ExternalInput")
    yout_d = nc.dram_tensor("yout", [EPAD], F32, kind="ExternalOutput")

    xtab_d = nc.dram_tensor("xtab_scratch", [NPAD, EMB], F16)

    with tile.TileContext(nc) as tc:
        with (
            tc.tile_pool(name="weights", bufs=1) as wpool,
            tc.tile_pool(name="gath", bufs=3) as gpool,
            tc.tile_pool(name="mid", bufs=2) as qpool,
            tc.tile_pool(name="smalls", bufs=1) as spool,
            tc.tile_pool(name="psVP", bufs=2, space="PSUM") as psVP,
            tc.tile_pool(name="psCS", bufs=1, space="PSUM") as psCS,
            tc.tile_pool(name="psG8", bufs=1, space="PSUM") as psG8,
            tc.tile_pool(name="psFIN", bufs=1, space="PSUM") as psFIN,
        ):
            # ---- load weights / tables ----
            featT0 = wpool.tile([128, NPAD], F16, tag="featT0")
            featT1 = wpool.tile([128, NPAD], F16, tag="featT1")
            nc.sync.dma_start(featT0[:], featT_d[0:128, :])
            nc.sync.dma_start(featT1[:], featT_d[128:256, :])
            wenc0 = wpool.tile([128, EMB], F16, tag="wenc0")
            wenc1 = wpool.tile([128, EMB], F16, tag="wenc1")
            nc.sync.dma_start(wenc0[:], wenc_d[0:128, :])
            nc.sync.dma_start(wenc1[:], wenc_d[128:256, :])
            wx = wpool.tile([EMB, CONV], F16, tag="wx")
            nc.sync.dma_start(wx[:], wx_d[:])
            wcs = wpool.tile([EMB, CONV], F16, tag="wcs")
            nc.sync.dma_start(wcs[:], wcs_d[:])
            wo = wpool.tile([CONV, 4], F16, tag="wo")
            nc.sync.dma_start(wo[:], wo_d[:])
            eyef = wpool.tile([128, 128], F16, tag="eyef")
            nc.sync.dma_start(eyef[:], eyef_d[:])
            vecs = wpool.tile([128, 8], F32, tag="vecs")
            nc.sync.dma_start(vecs[:], vecs_d[:])
            idx = wpool.tile([128, NIDXCOL], I16, tag="idx")
            nc.sync.dma_start(idx[:], idx_d[:])

            cconv = vecs[:, 3:4]   # c_const (+cheb_b) per CONV feature
            boutv = vecs[0:1, 4:5]  # b_out scalar
            bencv = vecs[:, 5:6]   # encoder bias per EMB feature
            v6 = vecs[:, 6:7]      # -(2s - s^2)/8

            # ---- encoder (feature-major) + transpose to node-major table ----
            xef = wpool.tile([128, NPAD], F16, tag="xef")
            for g in range(4):
                ep = psVP.tile([128, 512], F32, tag="vp", name="ep")
                nc.tensor.matmul(ep[:], wenc0[:], featT0[:, bass.ts(g, 512)],
                                 start=True, stop=False)
                nc.tensor.matmul(ep[:], wenc1[:], featT1[:, bass.ts(g, 512)],
                                 start=False, stop=True)
                eb = spool.tile([128, 512], F16, tag="eb", name=f"eb{g}",
                                bufs=2)
                nc.scalar.activation(eb[:], ep[:], AF.Identity, bias=bencv)
                nc.vector.tensor_scalar(xef[:, bass.ts(g, 512)], eb[:],
                                        1.0, -1.0, op0=OP.min, op1=OP.max)
            # transpose to node-major (4 chunks per PSUM tile) and store rows
            for g in range(4):
                xtp = psVP.tile([128, 4, 128], F16, tag="vp", name=f"xtp{g}")
                for t4 in range(4):
                    nc.tensor.transpose(xtp[:, t4, :],
                                        xef[:, bass.ts(4 * g + t4, 128)],
                                        eyef[:])
                xnm = spool.tile([128, 4, 128], F16, tag="xnm",
                                 name=f"xnm{g}", bufs=2)
                nc.scalar.activation(xnm[:].rearrange("p t e -> p (t e)"),
                                     xtp[:].rearrange("p t e -> p (t e)"),
                                     AF.Identity)
                r0 = g * 512
                nc.sync.dma_start(
                    xtab_d[r0:r0 + 512, :].rearrange("(t p) e -> p t e",
                                                     p=128),
                    xnm[:])

            logit = wpool.tile([1, EPAD], F32, tag="logit")
            gq = 0  # global gather counter for queue round-robin

            for b in range(NBLK):
                # ---- gather 4096 incidence rows, feature-major ----
                xT = gpool.tile([128, COLS], F16, tag="xT", name=f"xT{b}")
                for j in range(S):
                    gslice = xT[:, j * LB:(j + 1) * LB].unsqueeze(1)
                    icol = b * (COLS // 16) + j * (LB // 16)
                    nc.gpsimd.dma_gather(
                        gslice, xtab_d[:], idx[:, icol:icol + LB // 16],
                        LB, LB, EMB, transpose=True, queue_num=gq % 4)
                    gq += 1

                xpl = xT[:].rearrange("p (j e) -> p j e", j=S)

                # ---- xsq = x^2 (scalar); g8/q8 via PE identity-accumulate ----
                xsq = qpool.tile([128, S, LB], F16, tag="xsq", name=f"xsq{b}")
                nc.scalar.activation(xsq[:].rearrange("p j e -> p (j e)"),
                                     xT[:], AF.Square)
                gp = psG8.tile([128, LB], F32, tag="g8")
                for j in range(S):
                    nc.tensor.matmul(gp[:], eyef[:], xpl[:, j, :],
                                     start=(j == 0), stop=(j == S - 1))
                qp = psG8.tile([128, LB], F32, tag="q8")
                for j in range(S):
                    nc.tensor.matmul(qp[:], eyef[:], xsq[:, j, :],
                                     start=(j == 0), stop=(j == S - 1))

                # ---- GraphNorm scale (fp32 chain): ex = rsqrt(var+eps) ----
                t1 = spool.tile([128, LB], F32, tag="t1")
                nc.scalar.activation(t1[:], gp[:], AF.Square)
                vx8 = spool.tile([128, LB], F32, tag="vx8")
                nc.vector.scalar_tensor_tensor(vx8[:], t1[:], v6, qp[:],
                                               op0=OP.mult, op1=OP.add)
                vc = spool.tile([128, LB], F32, tag="vc")
                nc.vector.tensor_scalar(vc[:], vx8[:], 0.0, 8.0 * EPS,
                                        op0=OP.max, op1=OP.add)
                ex = spool.tile([128, LB], F16, tag="ex")
                nc.scalar.activation(ex[:], vc[:], AF.Abs_reciprocal_sqrt,
                                     scale=0.125)
                w8 = spool.tile([128, LB], F16, tag="w8")
                nc.vector.tensor_tensor(w8[:], ex[:], gp[:], op=OP.mult)

                # ---- per-hyperedge C = w8 @ Wcs (+cconv) ----
                cp = psCS.tile([128, LB], F32, tag="cs", name=f"cp{b}")
                nc.tensor.matmul(cp[:], wcs[:], w8[:], start=True, stop=True)
                cs = spool.tile([128, LB], F16, tag="cs16")
                nc.scalar.activation(cs[:], cp[:], AF.Identity, bias=cconv)

                # ---- rhs = x * ex (broadcast over planes) ----
                rhs = gpool.tile([128, S, LB], F16, tag="rhs", name=f"rhs{b}",
                                 bufs=2)
                nc.vector.tensor_tensor(
                    rhs[:], xpl,
                    ex[:].unsqueeze(1).broadcast_to([128, S, LB]), op=OP.mult)

                # ---- cheb conv: 2 superwaves of 4 planes (cs prefilled,
                # ldweights shared across plane-pairs) ----
                z2 = qpool.tile([128, S, LB], F16, tag="z2", name=f"z2_{b}")
                for sw in range(2):
                    vps = []
                    for w in range(2):
                        vp = psVP.tile([128, 2, LB], F32, tag="vp",
                                       name=f"vp{b}_{sw}_{w}")
                        nc.tensor.matmul(vp[:, 0, :], eyef[:], cs[:],
                                         start=True, stop=False)
                        nc.tensor.matmul(vp[:, 1, :], eyef[:], cs[:],
                                         start=True, stop=False)
                        vps.append(vp)
                    for w in range(2):
                        for h in range(2):
                            j = 4 * sw + 2 * w + h
                            nc.tensor.matmul(vps[w][:, h, :], wx[:],
                                             rhs[:, j, :],
                                             start=False, stop=True)
                    for w in range(2):
                        j0 = 4 * sw + 2 * w
                        nc.scalar.activation(
                            z2[:, j0:j0 + 2, :].rearrange("p j e -> p (j e)"),
                            vps[w][:].rearrange("p j e -> p (j e)"),
                            AF.Identity)

                # ---- poolings: max/min/ssq trees (fp16) ----
                m1 = spool.tile([128, 4, LB], F16, tag="m1")
                nc.vector.tensor_tensor(m1[:], z2[:, 0:4, :], z2[:, 4:8, :],
                                        op=OP.max)
                m2 = spool.tile([128, 2, LB], F16, tag="m2")
                nc.vector.tensor_tensor(m2[:], m1[:, 0:2, :], m1[:, 2:4, :],
                                        op=OP.max)
                zmax = spool.tile([128, LB], F16, tag="zmax")
                nc.vector.tensor_tensor(zmax[:], m2[:, 0, :], m2[:, 1, :],
                                        op=OP.max)
                n1 = spool.tile([128, 4, LB], F16, tag="n1")
                nc.vector.tensor_tensor(n1[:], z2[:, 0:4, :], z2[:, 4:8, :],
                                        op=OP.min)
                n2 = spool.tile([128, 2, LB], F16, tag="n2")
                nc.vector.tensor_tensor(n2[:], n1[:, 0:2, :], n1[:, 2:4, :],
                                        op=OP.min)
                zmin = spool.tile([128, LB], F16, tag="zmin")
                nc.vector.tensor_tensor(zmin[:], n2[:, 0, :], n2[:, 1, :],
                                        op=OP.min)

                # z^2: split halves across scalar and vector engines
                zq = qpool.tile([128, S, LB], F16, tag="zq", name=f"zq{b}",
                                bufs=1)
                nc.scalar.activation(
                    zq[:, 0:4, :].rearrange("p j e -> p (j e)"),
                    z2[:, 0:4, :].rearrange("p j e -> p (j e)"), AF.Square)
                nc.vector.tensor_tensor(zq[:, 4:8, :], z2[:, 4:8, :],
                                        z2[:, 4:8, :], op=OP.mult)
                zqc = qpool.tile([128, S, LB], F16, tag="zqc", name=f"zqc{b}",
                                 bufs=1)
                nc.vector.tensor_scalar(zqc[:], zq[:], 1.0, None, op0=OP.min)
                s1 = spool.tile([128, 4, LB], F16, tag="s1")
                nc.vector.tensor_tensor(s1[:], zqc[:, 0:4, :], zqc[:, 4:8, :],
                                        op=OP.add)
                s2 = spool.tile([128, 2, LB], F16, tag="s2")
                nc.vector.tensor_tensor(s2[:], s1[:, 0:2, :], s1[:, 2:4, :],
                                        op=OP.add)
                ssq = spool.tile([128, LB], F16, tag="ssq")
                nc.vector.tensor_tensor(ssq[:], s2[:, 0, :], s2[:, 1, :],
                                        op=OP.add)

                # clip pooled extrema (exact: clip commutes with max/min)
                zmaxc = spool.tile([128, LB], F16, tag="zmaxc")
                nc.vector.tensor_scalar(zmaxc[:], zmax[:], 1.0, -1.0,
                                        op0=OP.min, op1=OP.max)
                zminc = spool.tile([128, LB], F16, tag="zminc")
                nc.vector.tensor_scalar(zminc[:], zmin[:], 1.0, -1.0,
                                        op0=OP.min, op1=OP.max)
                # ynorm = sqrt(ssq/8) = (ssq/8) * rsqrt(ssq/8); avoids the
                # Sqrt act-table (stays on the abs_rsqrt set all block long)
                r2 = spool.tile([128, LB], F32, tag="r2")
                nc.scalar.activation(r2[:], ssq[:], AF.Abs_reciprocal_sqrt,
                                     scale=0.125, bias=vecs[:, 7:8])
                ynorm = spool.tile([128, LB], F16, tag="ynorm")
                nc.vector.scalar_tensor_tensor(ynorm[:], ssq[:], 0.125, r2[:],
                                               op0=OP.mult, op1=OP.mult)

                # ---- logits: wo0@zmaxc - wo0@zminc + wo1@ynorm ----
                fp = psFIN.tile([1, LB], F32, tag="fin")
                nc.tensor.matmul(fp[:], wo[:, 0:1], zmaxc[:],
                                 start=True, stop=False)
                nc.tensor.matmul(fp[:], wo[:, 1:2], zminc[:],
                                 start=False, stop=False)
                nc.tensor.matmul(fp[:], wo[:, 2:3], ynorm[:],
                                 start=False, stop=True)
                nc.scalar.activation(logit[0:1, b * LB:(b + 1) * LB], fp[:],
                                     AF.Identity)

            ysb = wpool.tile([1, EPAD], F32, tag="ysb")
            nc.scalar.activation(ysb[:], logit[:], AF.Sigmoid, bias=boutv)
            nc.sync.dma_start(yout_d[:].rearrange("(p c) -> p c", p=1), ysb[:])

    nc.compile()
    return nc


def _get_program():
    if "nc" not in _CACHE:
        _CACHE["nc"] = _build_program()
    return _CACHE["nc"]


def _host_prep(inputs):
    """Fold weights and stage per-core input maps."""
    f = lambda k: np.asarray(inputs[k], np.float32)
    feature = f("feature")
    W_enc, b_enc = f("W_enc"), f("b_enc")
    gw, gb, gs = f("gn_weight"), f("gn_bias"), f("gn_mean_scale")
    cheb_W = np.asarray(inputs["cheb_W"], np.float64)
    cheb_b = np.asarray(inputs["cheb_b"], np.float64)
    W_out, b_out = f("W_out"), f("b_out")
    hn = np.asarray(inputs["hyperedge_nodes"]).astype(np.int64)

    d = float(S - 1)
    W0, W1, W2 = cheb_W[0], cheb_W[1], cheb_W[2]
    Wx64 = W0 + W1 / d + W2 * ((2.0 - d * d) / (d * d))
    Wg64 = -W1 / d + W2 * (2.0 * (d - 1.0) / (d * d))
    c_const = (gb.astype(np.float64) @ (Wx64 + S * Wg64) + cheb_b)
    gw64 = gw.astype(np.float64)[:, None]
    gs64 = gs.astype(np.float64)[:, None]
    # fold gn_weight into Wx; fold gn_weight * (u,w8) combo into one Wcs
    wx16 = (gw64 * Wx64).astype(np.float16)
    wcs16 = (gw64 * (gs64 / 8.0 * (-(Wx64 + S * Wg64)) + Wg64)).astype(
        np.float16)

    featT = np.zeros((F, NPAD), np.float16)
    featT[:, :N] = feature.T.astype(np.float16)
    wenc = W_enc.astype(np.float16)
    wo16 = np.zeros((CONV, 4), np.float16)
    wo16[:, 0] = W_out[:CONV, 0].astype(np.float16)
    wo16[:, 1] = (-W_out[:CONV, 0]).astype(np.float16)
    wo16[:, 2] = W_out[CONV:, 0].astype(np.float16)
    eyef = np.eye(128, dtype=np.float16)
    vecs = np.zeros((128, 8), np.float32)
    vecs[:, 3] = c_const.astype(np.float32)
    vecs[0, 4] = b_out[0]
    vecs[:, 5] = b_enc
    vecs[:, 6] = -(2.0 * gs - gs * gs) / 8.0
    vecs[:, 7] = 1e-30

    shared = dict(featT=featT, wenc=wenc, wx=wx16, wcs=wcs16, wo=wo16,
                  eyef=eyef, vecs=vecs)

    in_maps = []
    for c in range(NCORES):
        base = c * ECORE
        hpad = np.zeros((EPAD, S), np.int16)
        hpad[:ECORE] = hn[base:base + ECORE].astype(np.int16)
        # per block b: column i = j*LB + e -> node hpad[b*LB + e, j]
        unwrapped = hpad.reshape(NBLK, LB, S).transpose(0, 2, 1).reshape(-1)
        idx16 = np.zeros((128, NIDXCOL), np.int16)
        wrapped = unwrapped.reshape(NIDXCOL, 16).T  # [16, NIDXCOL]
        for r in range(8):
            idx16[16 * r:16 * r + 16] = wrapped
        in_maps.append(dict(shared, idx16=idx16))
    return in_maps


def _install_trace_hook():
    """Best-effort NTFF profiling under axon (test/benchmark only)."""
    import types
    ah = sys.modules.get("antenv.axon_hooks")
    if ah is None:
        ah = types.ModuleType("antenv.axon_hooks")
        ah._HOOK = None
        ah.set_axon_ntff_profile_hook = lambda h: setattr(ah, "_HOOK", h)
        ah.get_axon_ntff_profile_hook = lambda: ah._HOOK
        sys.modules["antenv.axon_hooks"] = ah
        import antenv
        antenv.axon_hooks = ah
    if ah.get_axon_ntff_profile_hook() is None:
        from trn_agent_boot.trn_boot import _ntff_profile_via_ctypes
        hook = _ntff_profile_via_ctypes("/opt/axon/libaxon_pjrt.so")
        if hook is not None:
            ah.set_axon_ntff_profile_hook(hook)
    import concourse.bass_utils as bu
    bu.upload_artifacts = lambda tmpdir: f"local:{tmpdir}"


def _run(in_maps, trace=False):
    nc = _get_program()
    if trace:
        _install_trace_hook()
    return run_bass_kernel_spmd(nc, in_maps, list(range(NCORES)), trace=trace)


def kernel(**inputs) -> np.ndarray:
    in_maps = _host_prep(inputs)
    res = _run(in_maps)
    out = np.concatenate([res.results[c]["yout"][:ECORE] for c in range(NCORES)])
    return out.reshape(E, 1).astype(np.float32)


def kernel_traced(**inputs):
    """Like kernel() but returns (output, exec_time_ns) using a profiled run."""
    in_maps = _host_prep(inputs)
    res = _run(in_maps, trace=True)
    out = np.concatenate([res.results[c]["yout"][:ECORE] for c in range(NCORES)])
    return out.reshape(E, 1).astype(np.float32), res.exec_time_ns


# revision 13
# speedup vs baseline: 1.1535x; 1.1535x over previous
"""Trainium2 Bass kernel for nn_CHESHIRE (hypergraph GNN message passing).

Strategy (hyperedge-parallel across the 8 cores):
  * Clique Laplacian over 8-node cliques collapses the K=3 Chebyshev conv to
    out = x_gn @ Wx + gsum(x_gn) @ Wg with host-folded weight combos; the
    GraphNorm affine is folded into the same matmuls, with gn_weight/
    gn_mean_scale folded into the weight matrices host-side so the per-edge
    scale is just ex = rsqrt(var+eps).
  * Node encodings are computed once per core and stored to DRAM as an fp16
    [node, x] table (256B rows); incidence rows are fetched with SWDGE
    dma_gather (transpose=True), which lands the data feature-major directly
    (no PE transposes), one 512-idx gather per member plane, round-robined
    over 4 SWDGE queues so descriptor rings never block back-to-back.
  * Per-edge sums (g8) accumulate over the 8 member planes in PSUM via
    identity matmuls; the stats chain runs in fp32 (variance cancellation);
    q8/ssq/max/min pool via fp16 tensor-tensor trees.
  * The per-edge ChebConv constant C is prefilled into PSUM (identity matmul)
    so the conv matmul accumulates on top; clip is applied after the pooling
    trees (exact: clip is monotone, and min(z^2,1) == clip(z)^2).
"""

import sys

sys.path.insert(0, "/opt/trn_rl_repo")

import numpy as np

import concourse.bacc as bacc
import concourse.bass as bass
import concourse.mybir as mybir
from concourse import tile
from concourse.bass_utils import run_bass_kernel_spmd

F16 = mybir.dt.float16
F32 = mybir.dt.float32
I16 = mybir.dt.int16
AF = mybir.ActivationFunctionType
OP = mybir.AluOpType

# Problem constants (hardcoded per contract).
N, F, EMB, CONV = 2000, 256, 128, 128
E, S = 20000, 8
NCORES = 8
ECORE = E // NCORES          # 2500
NBLK = 5
LB = 512                     # edges per block
EPAD = NBLK * LB             # 2560
COLS = S * LB                # 4096 gathered columns per block
NIDXCOL = EPAD * S // 16     # 1280 int16 idx columns per core
NPAD = 2048                  # padded node count
EPS = 1e-5

_CACHE = {}


def _build_program():
    nc = bacc.Bacc(None, target_bir_lowering=False, debug=False,
                   num_swdge_queues=4)

    featT_d = nc.dram_tensor("featT", [F, NPAD], F16, kind="ExternalInput")
    wenc_d = nc.dram_tensor("wenc", [F, EMB], F16, kind="ExternalInput")
    wx_d = nc.dram_tensor("wx", [EMB, CONV], F16, kind="ExternalInput")
    wcs_d = nc.dram_tensor("wcs", [EMB, CONV], F16, kind="ExternalInput")
    wo_d = nc.dram_tensor("wo", [CONV, 4], F16, kind="ExternalInput")
    eyef_d = nc.dram_tensor("eyef", [128, 128], F16, kind="ExternalInput")
    vecs_d = nc.dram_tensor("vecs", [128, 8], F32, kind="ExternalInput")
    idx_d = nc.dram_tensor("idx16", [128, NIDXCOL], I16, kind="ExternalInput")
    yout_d = nc.dram_tensor("yout", [EPAD], F32, kind="ExternalOutput")

    xtab_d = nc.dram_tensor("xtab_scratch", [NPAD, EMB], F16)

    with tile.TileContext(nc) as tc:
        with (
            tc.tile_pool(name="weights", bufs=1) as wpool,
            tc.tile_pool(name="gath", bufs=3) as gpool,
            tc.tile_pool(name="mid", bufs=2) as qpool,
            tc.tile_pool(name="smalls", bufs=1) as spool,
            tc.tile_pool(name="psVP", bufs=2, space="PSUM") as psVP,
            tc.tile_pool(name="psCS", bufs=1, space="PSUM") as psCS,
            tc.tile_pool(name="psG8", bufs=1, space="PSUM") as psG8,
            tc.tile_pool(name="psFIN", bufs=1, space="PSUM") as psFIN,
        ):
            # ---- load weights / tables ----
            featT0 = wpool.tile([128, NPAD], F16, tag="featT0")
            featT1 = wpool.tile([128, NPAD], F16, tag="featT1")
            nc.sync.dma_start(featT0[:], featT_d[0:128, :])
            nc.sync.dma_start(featT1[:], featT_d[128:256, :])
            wenc0 = wpool.tile([128, EMB], F16, tag="wenc0")
            wenc1 = wpool.tile([128, EMB], F16, tag="wenc1")
            nc.sync.dma_start(wenc0[:], wenc_d[0:128, :])
            nc.sync.dma_start(wenc1[:], wenc_d[128:256, :])
            wx = wpool.tile([EMB, CONV], F16, tag="wx")
            nc.sync.dma_start(wx[:], wx_d[:])
            wcs = wpool.tile([EMB, CONV], F16, tag="wcs")
            nc.sync.dma_start(wcs[:], wcs_d[:])
            wo = wpool.tile([CONV, 4], F16, tag="wo")
            nc.sync.dma_start(wo[:], wo_d[:])
            eyef = wpool.tile([128, 128], F16, tag="eyef")
            nc.sync.dma_start(eyef[:], eyef_d[:])
            vecs = wpool.tile([128, 8], F32, tag="vecs")
            nc.sync.dma_start(vecs[:], vecs_d[:])
            idx = wpool.tile([128, NIDXCOL], I16, tag="idx")
            nc.sync.dma_start(idx[:], idx_d[:])

            cconv = vecs[:, 3:4]   # c_const (+cheb_b) per CONV feature
            boutv = vecs[0:1, 4:5]  # b_out scalar
            bencv = vecs[:, 5:6]   # encoder bias per EMB feature
            v6 = vecs[:, 6:7]      # -(2s - s^2)/8

            # ---- encoder (feature-major) + transpose to node-major table ----
            xef = wpool.tile([128, NPAD], F16, tag="xef")
            for g in range(4):
                ep = psVP.tile([128, 512], F32, tag="vp", name="ep")
                nc.tensor.matmul(ep[:], wenc0[:], featT0[:, bass.ts(g, 512)],
                                 start=True, stop=False)
                nc.tensor.matmul(ep[:], wenc1[:], featT1[:, bass.ts(g, 512)],
                                 start=False, stop=True)
                eb = spool.tile([128, 512], F16, tag="eb", name=f"eb{g}",
                                bufs=2)
                nc.scalar.activation(eb[:], ep[:], AF.Identity, bias=bencv)
                nc.vector.tensor_scalar(xef[:, bass.ts(g, 512)], eb[:],
                                        1.0, -1.0, op0=OP.min, op1=OP.max)
            # transpose to node-major (4 chunks per PSUM tile) and store rows
            for g in range(4):
                xtp = psVP.tile([128, 4, 128], F16, tag="vp", name=f"xtp{g}")
                for t4 in range(4):
                    nc.tensor.transpose(xtp[:, t4, :],
                                        xef[:, bass.ts(4 * g + t4, 128)],
                                        eyef[:])
                xnm = spool.tile([128, 4, 128], F16, tag="xnm",
                                 name=f"xnm{g}", bufs=2)
                nc.scalar.activation(xnm[:].rearrange("p t e -> p (t e)"),
                                     xtp[:].rearrange("p t e -> p (t e)"),
                                     AF.Identity)
                r0 = g * 512
                nc.sync.dma_start(
                    xtab_d[r0:r0 + 512, :].rearrange("(t p) e -> p t e",
                                                     p=128),
                    xnm[:])

            logit = wpool.tile([1, EPAD], F32, tag="logit")

            def stage_gather(b):
                """S1: gather 4096 incidence rows, feature-major."""
                xT = gpool.tile([128, COLS], F16, tag="xT", name=f"xT{b}")
                for j in range(S):
                    gslice = xT[:, j * LB:(j + 1) * LB].unsqueeze(1)
                    icol = b * (COLS // 16) + j * (LB // 16)
                    nc.gpsimd.dma_gather(
                        gslice, xtab_d[:], idx[:, icol:icol + LB // 16],
                        LB, LB, EMB, transpose=True,
                        queue_num=(b * S + j) % 4)
                return xT

            def stage_stats(b, xT):
                """S2: stats + GraphNorm scale + per-edge C + rhs."""
                xpl = xT[:].rearrange("p (j e) -> p j e", j=S)
                xsq = qpool.tile([128, S, LB], F16, tag="xsq", name=f"xsq{b}")
                nc.scalar.activation(xsq[:].rearrange("p j e -> p (j e)"),
                                     xT[:], AF.Square)
                gp = psG8.tile([128, LB], F32, tag="g8")
                for j in range(S):
                    nc.tensor.matmul(gp[:], eyef[:], xpl[:, j, :],
                                     start=(j == 0), stop=(j == S - 1))
                qp = psG8.tile([128, LB], F32, tag="q8")
                for j in range(S):
                    nc.tensor.matmul(qp[:], eyef[:], xsq[:, j, :],
                                     start=(j == 0), stop=(j == S - 1))

                t1 = spool.tile([128, LB], F32, tag="t1")
                nc.scalar.activation(t1[:], gp[:], AF.Square)
                vx8 = spool.tile([128, LB], F32, tag="vx8")
                nc.vector.scalar_tensor_tensor(vx8[:], t1[:], v6, qp[:],
                                               op0=OP.mult, op1=OP.add)
                vc = spool.tile([128, LB], F32, tag="vc")
                nc.vector.tensor_scalar(vc[:], vx8[:], 0.0, 8.0 * EPS,
                                        op0=OP.max, op1=OP.add)
                ex = spool.tile([128, LB], F16, tag="ex", bufs=2)
                nc.scalar.activation(ex[:], vc[:], AF.Abs_reciprocal_sqrt,
                                     scale=0.125)
                w8 = spool.tile([128, LB], F16, tag="w8")
                nc.vector.tensor_tensor(w8[:], ex[:], gp[:], op=OP.mult)

                cp = psCS.tile([128, LB], F32, tag="cs", name=f"cp{b}")
                nc.tensor.matmul(cp[:], wcs[:], w8[:], start=True, stop=True)
                cs = spool.tile([128, LB], F16, tag="cs16", bufs=2)
                nc.scalar.activation(cs[:], cp[:], AF.Identity, bias=cconv)

                rhs = gpool.tile([128, S, LB], F16, tag="rhs", name=f"rhs{b}",
                                 bufs=2)
                nc.vector.tensor_tensor(
                    rhs[:], xpl,
                    ex[:].unsqueeze(1).broadcast_to([128, S, LB]), op=OP.mult)
                return cs, rhs

            def stage_z(b, cs, rhs):
                """S3: cheb conv waves + poolings + logits."""
                z2 = qpool.tile([128, S, LB], F16, tag="z2", name=f"z2_{b}")
                for sw in range(2):
                    vps = []
                    for w in range(2):
                        vp = psVP.tile([128, 2, LB], F32, tag="vp",
                                       name=f"vp{b}_{sw}_{w}")
                        nc.tensor.matmul(vp[:, 0, :], eyef[:], cs[:],
                                         start=True, stop=False)
                        nc.tensor.matmul(vp[:, 1, :], eyef[:], cs[:],
                                         start=True, stop=False)
                        vps.append(vp)
                    for w in range(2):
                        for h in range(2):
                            j = 4 * sw + 2 * w + h
                            nc.tensor.matmul(vps[w][:, h, :], wx[:],
                                             rhs[:, j, :],
                                             start=False, stop=True)
                    for w in range(2):
                        j0 = 4 * sw + 2 * w
                        nc.scalar.activation(
                            z2[:, j0:j0 + 2, :].rearrange("p j e -> p (j e)"),
                            vps[w][:].rearrange("p j e -> p (j e)"),
                            AF.Identity)

                # ---- poolings: max/min/ssq trees (fp16) ----
                m1 = spool.tile([128, 4, LB], F16, tag="m1")
                nc.vector.tensor_tensor(m1[:], z2[:, 0:4, :], z2[:, 4:8, :],
                                        op=OP.max)
                m2 = spool.tile([128, 2, LB], F16, tag="m2")
                nc.vector.tensor_tensor(m2[:], m1[:, 0:2, :], m1[:, 2:4, :],
                                        op=OP.max)
                zmax = spool.tile([128, LB], F16, tag="zmax")
                nc.vector.tensor_tensor(zmax[:], m2[:, 0, :], m2[:, 1, :],
                                        op=OP.max)
                n1 = spool.tile([128, 4, LB], F16, tag="n1")
                nc.vector.tensor_tensor(n1[:], z2[:, 0:4, :], z2[:, 4:8, :],
                                        op=OP.min)
                n2 = spool.tile([128, 2, LB], F16, tag="n2")
                nc.vector.tensor_tensor(n2[:], n1[:, 0:2, :], n1[:, 2:4, :],
                                        op=OP.min)
                zmin = spool.tile([128, LB], F16, tag="zmin")
                nc.vector.tensor_tensor(zmin[:], n2[:, 0, :], n2[:, 1, :],
                                        op=OP.min)

                # z^2: split halves across scalar and vector engines
                zq = qpool.tile([128, S, LB], F16, tag="zq", name=f"zq{b}",
                                bufs=1)
                nc.scalar.activation(
                    zq[:, 0:4, :].rearrange("p j e -> p (j e)"),
                    z2[:, 0:4, :].rearrange("p j e -> p (j e)"), AF.Square)
                nc.vector.tensor_tensor(zq[:, 4:8, :], z2[:, 4:8, :],
                                        z2[:, 4:8, :], op=OP.mult)
                zqc = qpool.tile([128, S, LB], F16, tag="zqc", name=f"zqc{b}",
                                 bufs=1)
                nc.vector.tensor_scalar(zqc[:], zq[:], 1.0, None, op0=OP.min)
                s1 = spool.tile([128, 4, LB], F16, tag="s1")
                nc.vector.tensor_tensor(s1[:], zqc[:, 0:4, :], zqc[:, 4:8, :],
                                        op=OP.add)
                s2 = spool.tile([128, 2, LB], F16, tag="s2")
                nc.vector.tensor_tensor(s2[:], s1[:, 0:2, :], s1[:, 2:4, :],
                                        op=OP.add)
                ssq = spool.tile([128, LB], F16, tag="ssq")
                nc.vector.tensor_tensor(ssq[:], s2[:, 0, :], s2[:, 1, :],
                                        op=OP.add)

                # clip pooled extrema (exact: clip commutes with max/min)
                zmaxc = spool.tile([128, LB], F16, tag="zmaxc")
                nc.vector.tensor_scalar(zmaxc[:], zmax[:], 1.0, -1.0,
                                        op0=OP.min, op1=OP.max)
                zminc = spool.tile([128, LB], F16, tag="zminc")
                nc.vector.tensor_scalar(zminc[:], zmin[:], 1.0, -1.0,
                                        op0=OP.min, op1=OP.max)
                # ynorm = sqrt(ssq/8) = (ssq/8) * rsqrt(ssq/8); avoids the
                # Sqrt act-table (stays on the abs_rsqrt set all block long)
                r2 = spool.tile([128, LB], F32, tag="r2")
                nc.scalar.activation(r2[:], ssq[:], AF.Abs_reciprocal_sqrt,
                                     scale=0.125, bias=vecs[:, 7:8])
                ynorm = spool.tile([128, LB], F16, tag="ynorm")
                nc.vector.scalar_tensor_tensor(ynorm[:], ssq[:], 0.125, r2[:],
                                               op0=OP.mult, op1=OP.mult)

                # ---- logits: wo0@zmaxc - wo0@zminc + wo1@ynorm ----
                fp = psFIN.tile([1, LB], F32, tag="fin")
                nc.tensor.matmul(fp[:], wo[:, 0:1], zmaxc[:],
                                 start=True, stop=False)
                nc.tensor.matmul(fp[:], wo[:, 1:2], zminc[:],
                                 start=False, stop=False)
                nc.tensor.matmul(fp[:], wo[:, 2:3], ynorm[:],
                                 start=False, stop=True)
                nc.scalar.activation(logit[0:1, b * LB:(b + 1) * LB], fp[:],
                                     AF.Identity)

            # software-pipelined emission: S1(b) | S2(b-1) | S3(b-2)
            st = {}
            for t in range(NBLK + 2):
                if t < NBLK:
                    st[t] = [stage_gather(t), None]
                if 1 <= t <= NBLK:
                    st[t - 1][1] = stage_stats(t - 1, st[t - 1][0])
                if t >= 2:
                    cs_rhs = st[t - 2][1]
                    stage_z(t - 2, *cs_rhs)
                    del st[t - 2]

            ysb = wpool.tile([1, EPAD], F32, tag="ysb")
            nc.scalar.activation(ysb[:], logit[:], AF.Sigmoid, bias=boutv)
            nc.sync.dma_start(yout_d[:].rearrange("(p c) -> p c", p=1), ysb[:])

    nc.compile()
    return nc


def _get_program():
    if "nc" not in _CACHE:
        _CACHE["nc"] = _build_program()
    return _CACHE["nc"]


def _host_prep(inputs):
    """Fold weights and stage per-core input maps."""
    f = lambda k: np.asarray(inputs[k], np.float32)
    feature = f("feature")
    W_enc, b_enc = f("W_enc"), f("b_enc")
    gw, gb, gs = f("gn_weight"), f("gn_bias"), f("gn_mean_scale")
    cheb_W = np.asarray(inputs["cheb_W"], np.float64)
    cheb_b = np.asarray(inputs["cheb_b"], np.float64)
    W_out, b_out = f("W_out"), f("b_out")
    hn = np.asarray(inputs["hyperedge_nodes"]).astype(np.int64)

    d = float(S - 1)
    W0, W1, W2 = cheb_W[0], cheb_W[1], cheb_W[2]
    Wx64 = W0 + W1 / d + W2 * ((2.0 - d * d) / (d * d))
    Wg64 = -W1 / d + W2 * (2.0 * (d - 1.0) / (d * d))
    c_const = (gb.astype(np.float64) @ (Wx64 + S * Wg64) + cheb_b)
    gw64 = gw.astype(np.float64)[:, None]
    gs64 = gs.astype(np.float64)[:, None]
    # fold gn_weight into Wx; fold gn_weight * (u,w8) combo into one Wcs
    wx16 = (gw64 * Wx64).astype(np.float16)
    wcs16 = (gw64 * (gs64 / 8.0 * (-(Wx64 + S * Wg64)) + Wg64)).astype(
        np.float16)

    featT = np.zeros((F, NPAD), np.float16)
    featT[:, :N] = feature.T.astype(np.float16)
    wenc = W_enc.astype(np.float16)
    wo16 = np.zeros((CONV, 4), np.float16)
    wo16[:, 0] = W_out[:CONV, 0].astype(np.float16)
    wo16[:, 1] = (-W_out[:CONV, 0]).astype(np.float16)
    wo16[:, 2] = W_out[CONV:, 0].astype(np.float16)
    eyef = np.eye(128, dtype=np.float16)
    vecs = np.zeros((128, 8), np.float32)
    vecs[:, 3] = c_const.astype(np.float32)
    vecs[0, 4] = b_out[0]
    vecs[:, 5] = b_enc
    vecs[:, 6] = -(2.0 * gs - gs * gs) / 8.0
    vecs[:, 7] = 1e-30

    shared = dict(featT=featT, wenc=wenc, wx=wx16, wcs=wcs16, wo=wo16,
                  eyef=eyef, vecs=vecs)

    in_maps = []
    for c in range(NCORES):
        base = c * ECORE
        hpad = np.zeros((EPAD, S), np.int16)
        hpad[:ECORE] = hn[base:base + ECORE].astype(np.int16)
        # per block b: column i = j*LB + e -> node hpad[b*LB + e, j]
        unwrapped = hpad.reshape(NBLK, LB, S).transpose(0, 2, 1).reshape(-1)
        idx16 = np.zeros((128, NIDXCOL), np.int16)
        wrapped = unwrapped.reshape(NIDXCOL, 16).T  # [16, NIDXCOL]
        for r in range(8):
            idx16[16 * r:16 * r + 16] = wrapped
        in_maps.append(dict(shared, idx16=idx16))
    return in_maps


def _install_trace_hook():
    """Best-effort NTFF profiling under axon (test/benchmark only)."""
    import types
    ah = sys.modules.get("antenv.axon_hooks")
    if ah is None:
        ah = types.ModuleType("antenv.axon_hooks")
        ah._HOOK = None
        ah.set_axon_ntff_profile_hook = lambda h: setattr(ah, "_HOOK", h)
        ah.get_axon_ntff_profile_hook = lambda: ah._HOOK
        sys.modules["antenv.axon_hooks"] = ah
        import antenv
        antenv.axon_hooks = ah
    if ah.get_axon_ntff_profile_hook() is None:
        from trn_agent_boot.trn_boot import _ntff_profile_via_ctypes
        hook = _ntff_profile_via_ctypes("/opt/axon/libaxon_pjrt.so")
        if hook is not None:
            ah.set_axon_ntff_profile_hook(hook)
    import concourse.bass_utils as bu
    bu.upload_artifacts = lambda tmpdir: f"local:{tmpdir}"


def _run(in_maps, trace=False):
    nc = _get_program()
    if trace:
        _install_trace_hook()
    return run_bass_kernel_spmd(nc, in_maps, list(range(NCORES)), trace=trace)


def kernel(**inputs) -> np.ndarray:
    in_maps = _host_prep(inputs)
    res = _run(in_maps)
    out = np.concatenate([res.results[c]["yout"][:ECORE] for c in range(NCORES)])
    return out.reshape(E, 1).astype(np.float32)


def kernel_traced(**inputs):
    """Like kernel() but returns (output, exec_time_ns) using a profiled run."""
    in_maps = _host_prep(inputs)
    res = _run(in_maps, trace=True)
    out = np.concatenate([res.results[c]["yout"][:ECORE] for c in range(NCORES)])
    return out.reshape(E, 1).astype(np.float32), res.exec_time_ns
